# revision 57
# baseline (speedup 1.0000x reference)
"""Trainium2 Bass kernel for multi-head attention (B=8, N=1024, C=768, H=12).

Strategy: data-parallel over batch — core b computes batch element b entirely
locally (no collectives). Host prepares transposed bf16 inputs; device does
QKV^T, scores S[k,q] (softmax without max-subtraction — scores ~N(0,1), exp is
safe), exp on ACT directly from PSUM, attn@V with the stationary laid out as
[ones x64 | V x64] so the softmax denominators land replicated on psum
partitions 0-63 (matmul cost is moving-free-dim rows only, so the replicas
are free). Softmax division then stays off the PE: full-lane DVE reciprocal
of the denominator rows + DVE multiply into the c-major attf layout consumed
by the output projection. Head pairs sit in partition halves 0/64 so the
K=64 score matmuls co-execute on disjoint PE row-groups (auto tile_position).
"""

import numpy as np
import ml_dtypes

B, N, C = 8, 1024, 768
H, HD = 12, 64
SCALE = HD ** -0.5
CT = C // 128   # 6 c-tiles
NT = N // 128   # 8 seq tiles
QB = 2          # q blocks of 512
PAIRS = H // 2  # 6 head pairs


def build_nc():
    import concourse.bass as bass
    import concourse.mybir as mybir
    import concourse.tile as tile
    from concourse import bacc
    from contextlib import ExitStack

    BF = mybir.dt.bfloat16
    F32 = mybir.dt.float32
    EXP = mybir.ActivationFunctionType.Exp

    nc = bacc.Bacc()
    xT = nc.declare_dram_parameter("xT", [C, N], BF, isOutput=False)
    wqkA = nc.declare_dram_parameter("wqkA", [C, 512], BF, isOutput=False)
    wqkB = nc.declare_dram_parameter("wqkB", [C, 1024], BF, isOutput=False)
    wvT = nc.declare_dram_parameter("wvT", [C, C], BF, isOutput=False)
    wpT = nc.declare_dram_parameter("wpT", [C, C], BF, isOutput=False)
    out = nc.declare_dram_parameter("out", [N, C], BF, isOutput=True)

    with tile.TileContext(nc, pool_alloc_mode="queue") as tc, ExitStack() as ctx:
        sb = ctx.enter_context(tc.tile_pool(name="sb", bufs=1))
        ptp = ctx.enter_context(tc.tile_pool(name="pt", bufs=32))
        yp = ctx.enter_context(tc.tile_pool(name="y", bufs=2))
        bcp = ctx.enter_context(tc.tile_pool(name="bcp", bufs=2))
        pbig = ctx.enter_context(tc.tile_pool(name="pbig", bufs=3, space="PSUM"))
        psmall = ctx.enter_context(tc.tile_pool(name="psmall", bufs=2, space="PSUM"))

        # ---- persistent SBUF tiles
        xT_sb = sb.tile([128, CT * N], BF, tag="xT")            # ct at cols ct*1024
        wqk_sb = sb.tile([128, CT * 1536], BF, tag="wqk")       # ct at cols ct*1536
        wv_sb = sb.tile([128, CT * C], BF, tag="wv")            # ct at cols ct*768
        wp_sb = sb.tile([128, CT * C], BF, tag="wp")
        qkT_sb = sb.tile([128, 12 * N], BF, tag="qkT")          # m-tile mt at cols mt*1024
        # kt at cols kt*1536; head h at +h*128: [ones x64 | V_h x64]. The 64
        # replicated ones-columns make attn@V emit the softmax denominator on
        # psum partitions 0-63, so the reciprocal runs full-lane on DVE with
        # no partition broadcast.
        ves_sb = sb.tile([128, NT * 12 * 128], BF, tag="ves")
        attf_sb = sb.tile([128, CT * N], BF, tag="attf")        # divided attn output, c-major layout

        VS = 12 * 128  # 1536 cols per kt block in ves

        # ---- DMA inputs (xT/wqkA interleaved per c-tile so the first QKV
        # accumulation step can start after ~2 DMAs; wvT before wqkB since
        # v_ntile consumes it during pair 0, before the pair-1 qk feed)
        for ct in range(CT):
            nc.sync.dma_start(out=xT_sb[:, ct * N:(ct + 1) * N], in_=xT[ct * 128:(ct + 1) * 128, :])
            nc.sync.dma_start(out=wqk_sb[:, ct * 1536: ct * 1536 + 512], in_=wqkA[ct * 128:(ct + 1) * 128, :])
        for ct in range(CT):
            nc.sync.dma_start(out=wv_sb[:, ct * C:(ct + 1) * C], in_=wvT[ct * 128:(ct + 1) * 128, :])
        for ct in range(CT):
            nc.sync.dma_start(out=wqk_sb[:, ct * 1536 + 512:(ct + 1) * 1536], in_=wqkB[ct * 128:(ct + 1) * 128, :])
        for ct in range(CT):
            nc.sync.dma_start(out=wp_sb[:, ct * C:(ct + 1) * C], in_=wpT[ct * 128:(ct + 1) * 128, :])

        warm_sb = sb.tile([1, 16], F32, tag="warm")
        nc.gpsimd.memset(warm_sb[:, :], 0.0)
        nc.scalar.activation(warm_sb[:, :], warm_sb[:, :], EXP)  # preload exp table set

        # PE warm-up: ~3.5us of dummy matmuls during the input-DMA wait flips
        # the HAM clock gate (4096-cycle activity window) to full rate before
        # the first real matmuls arrive, which would otherwise run at 1.2 GHz
        wgarb = sb.tile([128, 256], BF, tag="wgarb")
        nc.gpsimd.memset(wgarb[:, :], 0.0)
        wps = pbig.tile([128, 512], F32, tag="big", name="warmps")
        for _ in range(16):
            nc.tensor.matmul(wps[:, 0:256], lhsT=wgarb[:, 0:128], rhs=wgarb[:, :],
                             start=True, stop=True)
        for kt in range(NT):
            vv = ves_sb[:, kt * VS:(kt + 1) * VS].rearrange("p (h e) -> p h e", e=128)
            nc.gpsimd.memset(vv[:, 0:12, 0:64], 1.0)

        # ---- helpers
        QK_ORD = [0, 6, 1, 7, 2, 8, 3, 9, 4, 10, 5, 11]

        def qk_mtile_half(mt, qb):
            ps = psmall.tile([128, 512], F32, tag="mm", name=f"qk{mt}_{qb}")
            for ct in range(CT):
                nc.tensor.matmul(
                    ps[:, :],
                    lhsT=wqk_sb[:, ct * 1536 + QK_ORD.index(mt) * 128: ct * 1536 + (QK_ORD.index(mt) + 1) * 128],
                    rhs=xT_sb[:, ct * N + qb * 512: ct * N + qb * 512 + 512],
                    start=(ct == 0), stop=(ct == CT - 1),
                )
            nc.vector.tensor_copy(qkT_sb[:, mt * N + qb * 512: mt * N + qb * 512 + 512], ps[:, :])

        def v_ntile(nt, pool, tag):
            """Compute V natural rows [nt*128, +128] and scatter into ves."""
            for vb in range(2):
                ps = pool.tile([128, 384], F32, tag=tag, name=f"v{nt}_{vb}")
                for ct in range(CT):
                    nc.tensor.matmul(
                        ps[:, :],
                        lhsT=xT_sb[:, ct * N + nt * 128: ct * N + (nt + 1) * 128],
                        rhs=wv_sb[:, ct * C + vb * 384: ct * C + (vb + 1) * 384],
                        start=(ct == 0), stop=(ct == CT - 1),
                    )
                dst = ves_sb[:, nt * VS:(nt + 1) * VS].rearrange("p (h e) -> p h e", e=128)
                nc.vector.tensor_copy(
                    dst[:, vb * 6:(vb + 1) * 6, 64:128],
                    ps[:, :].rearrange("p (h e) -> p h e", e=64),
                )

        def q_slice(h, qb):
            po = (h % 2) * 64
            return qkT_sb[po:po + 64, (h // 2) * N + qb * 512: (h // 2) * N + qb * 512 + 512]

        def k_slice(h, kt):
            po = (h % 2) * 64
            base = (6 + h // 2) * N + kt * 128
            return qkT_sb[po:po + 64, base: base + 128]

        # pipeline state
        pt_kt = {}         # (pair, kt, j) -> [128, 1024] bf16 exp tile

        def scores_and_exp(p, kt):
            h0, h1 = 2 * p, 2 * p + 1
            ps0 = pbig.tile([128, 1024], F32, tag="big")
            ps1 = pbig.tile([128, 1024], F32, tag="big")
            # alternate which head leads per kt so the psum-rotation wait
            # lands on the head whose tile frees first
            ha, pa, hb, pb = (h0, ps0, h1, ps1) if kt % 2 == 0 else (h1, ps1, h0, ps0)
            for qb in range(QB):
                nc.tensor.matmul(pa[:, qb * 512: qb * 512 + 512], lhsT=k_slice(ha, kt),
                                 rhs=q_slice(ha, qb), start=True, stop=True)
                nc.tensor.matmul(pb[:, qb * 512: qb * 512 + 512], lhsT=k_slice(hb, kt),
                                 rhs=q_slice(hb, qb), start=True, stop=True)
            pt0 = ptp.tile([128, 1024], BF, tag="pt", name=f"pt{p}_{kt}a")
            pt1 = ptp.tile([128, 1024], BF, tag="pt", name=f"pt{p}_{kt}b")
            pt_kt[(p, kt, 0)], pt_kt[(p, kt, 1)] = pt0, pt1
            # exp the lead head's tile first — it finishes first on the PE
            pta, ptb = (pt0, pt1) if kt % 2 == 0 else (pt1, pt0)
            nc.scalar.activation(pta[:, :], pa[:, :], EXP)
            nc.scalar.activation(ptb[:, :], pb[:, :], EXP)

        po_open = {}       # (p, j, qb) -> open psum accumulation tile
        po_done = {}       # (p, j, qb) -> closed psum tile awaiting divide

        def attn_burst_half(p, idx, pool=None, tag="mm"):
            """Half of an attn@V accumulation group (4 MMs); idx 0..7 walks
            (h0,qb0),(h0,qb1),(h1,qb0),(h1,qb1) two slots each. Group closes
            on the odd idx."""
            g = idx // 2
            j, qb = g // 2, g % 2
            h = 2 * p + j
            if idx % 2 == 0:
                pool = pool if pool is not None else psmall
                po_open[(p, j, qb)] = pool.tile([128, 512], F32, tag=tag, name=f"po{h}_{qb}")
            po = po_open[(p, j, qb)]
            k0 = (idx % 2) * 4
            for kt in range(k0, k0 + 4):
                nc.tensor.matmul(
                    po[:, :],
                    lhsT=ves_sb[:, kt * VS + h * 128: kt * VS + (h + 1) * 128],
                    rhs=pt_kt[(p, kt, j)][:, qb * 512: qb * 512 + 512],
                    start=(kt == 0), stop=(kt == NT - 1),
                )
            if idx % 2 == 1:
                po_done[(p, j, qb)] = po_open.pop((p, j, qb))
                if qb == 1:
                    for kt in range(NT):
                        pt_kt.pop((p, kt, j))

        def attn_burst(p, j, qb, pool=None, tag="mm"):
            attn_burst_half(p, (j * 2 + qb) * 2, pool=pool, tag=tag)
            attn_burst_half(p, (j * 2 + qb) * 2 + 1)

        def divide(p, j, qb):
            """Normalize one closed attn@V psum tile into attf (no PE):
            full-lane DVE recip of the denominator rows 0-63, DVE multiply of
            the V rows 64-127. (reciprocal_approx_fast ignores input partition
            offsets, hence den at partitions 0-63.)"""
            h = 2 * p + j
            po = po_done.pop((p, j, qb))
            rcp = bcp.tile([64, 512], F32, tag="bc", name=f"bc{h}_{qb}")
            nc.vector.reciprocal_approx_fast(rcp[:, :], po[0:64, :])
            prow = (h % 2) * 64
            nc.vector.tensor_mul(
                attf_sb[prow:prow + 64, (h // 2) * N + qb * 512: (h // 2) * N + qb * 512 + 512],
                po[64:128, :],
                rcp[:, :])

        # ---- emission schedule per pair p (8 kt slots):
        #   slots 0-3: scores/exp(p, kt) + attn@V group bursts of pair p-1,
        #              each group's divide chained right after it closes
        #              (psmall freed before slot 4)
        #   slots 4-7: qkT m-tile feed for pair p+1 (psmall reused);
        #              for the last pair, open its own j0 attn@V half-groups
        # startup: the four first qk m-tile halves emitted ct-major across
        # four concurrent psum tiles, so the in-order PE queue advances with
        # every (xT, wqkA) c-tile DMA instead of trickling one MM per tile
        su_specs = [(0, 0, psmall, "mm"), (6, 0, psmall, "mm"),
                    (0, 1, pbig, "big"), (6, 1, pbig, "big")]
        su_ps = [pool.tile([128, 512], F32, tag=tag, name=f"qk{mt}_{qb}")
                 for mt, qb, pool, tag in su_specs]
        for ct in range(CT):
            for (mt, qb, _, _), ps in zip(su_specs, su_ps):
                nc.tensor.matmul(
                    ps[:, :],
                    lhsT=wqk_sb[:, ct * 1536 + QK_ORD.index(mt) * 128: ct * 1536 + (QK_ORD.index(mt) + 1) * 128],
                    rhs=xT_sb[:, ct * N + qb * 512: ct * N + qb * 512 + 512],
                    start=(ct == 0), stop=(ct == CT - 1),
                )
        # split the four evacuations across DVE and ACT — serialized on one
        # engine they add ~2.5us before the first scores tile can start
        for i, ((mt, qb, _, _), ps) in enumerate(zip(su_specs, su_ps)):
            dstv = qkT_sb[:, mt * N + qb * 512: mt * N + qb * 512 + 512]
            if i % 2 == 0:
                nc.vector.tensor_copy(dstv, ps[:, :])
            else:
                nc.scalar.copy(dstv, ps[:, :])

        for p in range(PAIRS):
            for kt in range(NT):
                scores_and_exp(p, kt)
                if p == 0:
                    v_ntile(kt, psmall, "mm")
                if p >= 1 and kt < 4:
                    attn_burst(p - 1, kt // 2, kt % 2)
                    divide(p - 1, kt // 2, kt % 2)
                if kt >= 4:
                    if p + 1 < PAIRS:
                        mt, qb = [(p + 1, 0), (p + 1, 1), (p + 7, 0), (p + 7, 1)][kt - 4]
                        qk_mtile_half(mt, qb)
                    elif p == PAIRS - 1:
                        # last pair: trickle its (j0, qb) groups in as the exp
                        # tiles land, so the drain only owes kt7 + the j1 groups
                        if kt == 4:
                            attn_burst_half(p, 0)        # (j0,qb0) kt0-3
                        elif kt == 5:
                            attn_burst_half(p, 2)        # (j0,qb1) kt0-3
                        else:
                            kts = (4, 5) if kt == 6 else (6,)
                            for j0qb in (0, 1):
                                po = po_open[(p, 0, j0qb)]
                                for k in kts:
                                    nc.tensor.matmul(
                                        po[:, :],
                                        lhsT=ves_sb[:, k * VS + (2 * p) * 128: k * VS + (2 * p + 1) * 128],
                                        rhs=pt_kt[(p, k, 0)][:, j0qb * 512: j0qb * 512 + 512],
                                        start=False, stop=False,
                                    )

        # ---- drain: close last pair's groups, interleave divides and the
        # output projection so the PE stays busy through the tail
        open_groups = {}

        def open_proj(nt, mb, pool, tag):
            ps = pool.tile([128, 384], F32, tag=tag, name=f"y{nt}_{mb}")
            for ct in range(CT - 1):
                nc.tensor.matmul(
                    ps[:, :],
                    lhsT=attf_sb[:, ct * N + nt * 128: ct * N + (nt + 1) * 128],
                    rhs=wp_sb[:, ct * C + mb * 384: ct * C + (mb + 1) * 384],
                    start=(ct == 0), stop=False,
                )
            open_groups[(nt, mb)] = ps

        pl = PAIRS - 1
        open_proj(0, 0, pbig, "big")     # attf ct0-4 ready; fills the wait
        open_proj(0, 1, pbig, "big")     # for the last exp tiles
        for j0qb in (0, 1):              # close (j0, qb) groups: only kt7 left
            po = po_open.pop((pl, 0, j0qb))
            nc.tensor.matmul(
                po[:, :],
                lhsT=ves_sb[:, 7 * VS + (2 * pl) * 128: 7 * VS + (2 * pl + 1) * 128],
                rhs=pt_kt[(pl, 7, 0)][:, j0qb * 512: j0qb * 512 + 512],
                start=False, stop=True,
            )
            po_done[(pl, 0, j0qb)] = po
            divide(pl, 0, j0qb)
        for kt in range(NT):
            pt_kt.pop((pl, kt, 0))
        # j1's first group goes in the pbig slot freed by the last exp, so its
        # matmuls run while the j0 divides still hold the psmall slots
        attn_burst(pl, 1, 0, pool=pbig, tag="big")
        divide(pl, 1, 0)
        attn_burst(pl, 1, 1)
        open_proj(1, 0, psmall, "mm")
        divide(pl, 1, 1)
        open_proj(1, 1, psmall, "mm")

        # ---- output projection: y[n, m] = attf.T @ wpT (+ bias on host)
        for nt in range(NT):
            y_t = yp.tile([128, C], BF, tag="y")
            for mb in range(2):
                if (nt, mb) in open_groups:
                    ps = open_groups[(nt, mb)]
                    ct = CT - 1
                    nc.tensor.matmul(
                        ps[:, :],
                        lhsT=attf_sb[:, ct * N + nt * 128: ct * N + (nt + 1) * 128],
                        rhs=wp_sb[:, ct * C + mb * 384: ct * C + (mb + 1) * 384],
                        start=False, stop=True,
                    )
                else:
                    ps = psmall.tile([128, 384], F32, tag="mm", name=f"y{nt}_{mb}")
                    for ct in range(CT):
                        nc.tensor.matmul(
                            ps[:, :],
                            lhsT=attf_sb[:, ct * N + nt * 128: ct * N + (nt + 1) * 128],
                            rhs=wp_sb[:, ct * C + mb * 384: ct * C + (mb + 1) * 384],
                            start=(ct == 0), stop=(ct == CT - 1),
                        )
                if mb == 0:
                    nc.scalar.copy(y_t[:, mb * 384:(mb + 1) * 384], ps[:, :])
                else:
                    nc.vector.tensor_copy(y_t[:, mb * 384:(mb + 1) * 384], ps[:, :])
                if nt == NT - 1:
                    # last tile: per-half DMAs so the first transfer overlaps
                    # the second half's evacuation instead of waiting for both
                    nc.sync.dma_start(out=out[nt * 128:(nt + 1) * 128, mb * 384:(mb + 1) * 384],
                                      in_=y_t[:, mb * 384:(mb + 1) * 384])
            if nt < NT - 1:
                # one contiguous full-row DMA per n-tile (after both evacs)
                nc.sync.dma_start(out=out[nt * 128:(nt + 1) * 128, :], in_=y_t[:, :])

    nc.compile()
    return nc


_CACHE = {}


def _prep_inputs(x, w_qkv, w_proj):
    bf = ml_dtypes.bfloat16
    w = np.array(w_qkv, dtype=np.float32, copy=True)
    w[:C] *= SCALE
    wqkT = w[:2 * C].T.astype(bf)                                # [C, 2C]
    ord_ = [0, 6, 1, 7, 2, 8, 3, 9, 4, 10, 5, 11]
    wqkA = np.ascontiguousarray(np.concatenate([wqkT[:, mt * 128:(mt + 1) * 128] for mt in ord_[:4]], axis=1))
    wqkB = np.ascontiguousarray(np.concatenate([wqkT[:, mt * 128:(mt + 1) * 128] for mt in ord_[4:]], axis=1))
    wvT = np.ascontiguousarray(w[2 * C:].T.astype(bf))          # [C, C]
    wpT = np.ascontiguousarray(np.asarray(w_proj).T.astype(bf))  # [C, C]
    maps = []
    for b in range(B):
        maps.append({
            "xT": np.ascontiguousarray(np.asarray(x[b]).T.astype(bf)),
            "wqkA": wqkA, "wqkB": wqkB, "wvT": wvT, "wpT": wpT,
        })
    return maps


def kernel(x, w_qkv, w_proj, b_proj):
    from concourse.bass_utils import run_bass_kernel_spmd

    if "nc" not in _CACHE:
        _CACHE["nc"] = build_nc()
    nc = _CACHE["nc"]
    in_maps = _prep_inputs(x, w_qkv, w_proj)
    res = run_bass_kernel_spmd(nc, in_maps, core_ids=list(range(B)))
    y = np.stack([np.asarray(res.results[i]["out"], dtype=np.float32) for i in range(B)])
    y = y + np.asarray(b_proj, dtype=np.float32)[None, None, :]
    return y.astype(np.float32)


if __name__ == "__main__":
    nc = build_nc()
    print("build OK")
